# revision 2
# baseline (speedup 1.0000x reference)
"""CG coupler (segment_reduce) Trainium2 kernel.

out[b, ro[t]] += x1[b, r1[t]] * x2[b, r2[t]] * cg[t]   for t in range(T)

The CG index tables produced by the coupler have a rigid structure: T splits
into runs of exactly 128 consecutive indices (the channel dimension) that are
128-aligned in all three tensors, with a constant coefficient per run.  Each
run is therefore one dense slot-level FMA:

    out[:, so*128:(so+1)*128] += c * x1[:, s1*128:...] * x2[:, s2*128:...]

We detect that structure from the runtime index arrays on the host and bake it
into the Bass program.  Per core (batch is data-parallel across 8 cores):

  - inputs stream in per (pass, column-group) so products can start early
  - the distinct (s1,s2) slot products are computed in fp32, split between
    the DVE and Pool engines by a running load-balance
  - per-term scaled-identity matmuls accumulate into PSUM; operands are
    bitcast to float32r, which the PE runs at 1 cycle/row for moving size
    >= 256 (plain fp32 runs at 4 cycles/row)
  - matmuls for one output slot are issued contiguously (start on first,
    stop on last), so no PSUM-zeroing matmuls are needed
  - the Act engine evacuates each PSUM bank to SBUF; the bank's columns are
    then DMA'd straight to DRAM
"""

import sys

for _p in ("/opt/trn_rl_repo",):
    if _p not in sys.path:
        sys.path.insert(0, _p)

from contextlib import ExitStack

import numpy as np

import concourse.bass as bass
import concourse.mybir as mybir
import concourse.tile as tile
from concourse import bacc
from concourse.bass_utils import run_bass_kernel_spmd

N_CORES = 8
P = 128
F32 = mybir.dt.float32
F32R = mybir.dt.float32r

_CACHE: dict = {}


def _detect_plan(r1, r2, ro, cg, in_dim, out_dim):
    """Return list of (s1, s2, so, c) slot terms, or None if the index tables
    don't have the aligned 128-run structure."""
    T = len(cg)
    if T % P != 0 or len(r1) != T or len(r2) != T or len(ro) != T:
        return None
    d1 = np.diff(r1)
    d2 = np.diff(r2)
    do = np.diff(ro)
    brk = np.where(~((d1 == 1) & (d2 == 1) & (do == 1)))[0] + 1
    starts = np.concatenate([[0], brk])
    ends = np.concatenate([brk, [T]])
    if not np.all(ends - starts == P):
        return None
    a0, b0, o0 = r1[starts], r2[starts], ro[starts]
    if (a0 % P).any() or (b0 % P).any() or (o0 % P).any():
        return None
    if a0.max() + P > in_dim or b0.max() + P > in_dim or o0.max() + P > out_dim:
        return None
    cg2 = np.asarray(cg).reshape(-1, P)
    if not np.all(cg2 == cg2[:, :1]):
        return None
    return list(
        zip(
            (a0 // P).tolist(),
            (b0 // P).tolist(),
            (o0 // P).tolist(),
            cg2[:, 0].astype(np.float64).tolist(),
        )
    )


def _numpy_fallback(x1, x2, cg, r1, r2, ro, out_dim):
    out = np.zeros((x1.shape[0], out_dim), dtype=x1.dtype)
    prod = x1[:, r1] * x2[:, r2] * cg[None, :].astype(x1.dtype)
    np.add.at(out, (slice(None), ro), prod)
    return out


# cost-model engine-busy estimates (ns) for one [128, 256] tensor_tensor
_DVE_TT_NS = 327.0
_POOL_TT_NS = 508.0
_POOL_SETUP_NS = 290.0  # memset + affine_select per scaled identity

SLOTS_PER_GROUP = 4  # column-group granularity for input DMA (512 cols)


def _build_program(terms, b_shard, in_dim, out_dim):
    """Build the per-core Bass program. Every core runs the same program on
    its own batch shard (data-parallel, no collectives)."""
    nblk = b_shard // P
    assert nblk % 2 == 0
    n_passes = nblk // 2
    n_so = out_dim // P
    n_s_in = in_dim // P
    n_groups = (n_s_in + SLOTS_PER_GROUP - 1) // SLOTS_PER_GROUP
    gcols = SLOTS_PER_GROUP * P

    def grp(s):
        return s // SLOTS_PER_GROUP

    # distinct (s1, s2) pairs in data-readiness order
    pairs: dict = {}
    for s1, s2, so, c in terms:
        pairs.setdefault((s1, s2), []).append((so, c))
    pair_order = sorted(pairs, key=lambda p: (max(grp(p[0]), grp(p[1])), p))
    pair_ready = {p: i for i, p in enumerate(pair_order)}

    # output slots ordered by when their last pair product becomes available
    slot_terms: dict = {}
    for s1, s2, so, c in terms:
        slot_terms.setdefault(so, []).append(((s1, s2), c))
    slot_order = sorted(
        slot_terms, key=lambda so: (max(pair_ready[p] for p, _ in slot_terms[so]), so)
    )

    cvals = sorted({c for _, _, _, c in terms})

    nc = bacc.Bacc("TRN2", target_bir_lowering=False, debug=False)
    x1d = nc.dram_tensor("x1", [b_shard, in_dim], F32, kind="ExternalInput").ap()
    x2d = nc.dram_tensor("x2", [b_shard, in_dim], F32, kind="ExternalInput").ap()
    outd = nc.dram_tensor("out", [b_shard, out_dim], F32, kind="ExternalOutput").ap()

    with tile.TileContext(nc) as tc, ExitStack() as ctx:
        const_p = ctx.enter_context(tc.tile_pool(name="const", bufs=1))
        big_p = ctx.enter_context(tc.tile_pool(name="big", bufs=1))
        prod_p = ctx.enter_context(tc.tile_pool(name="prod", bufs=80))
        psum_p = ctx.enter_context(tc.tile_pool(name="psum", bufs=8, space="PSUM"))

        # scaled identity matrices, one per distinct CG coefficient (Pool)
        sids = {}
        for i, c in enumerate(cvals):
            t = const_p.tile([P, P], F32, tag=f"sid{i}")
            nc.gpsimd.memset(t[:], 0.0)
            nc.gpsimd.affine_select(
                out=t[:],
                in_=t[:],
                compare_op=mybir.AluOpType.not_equal,
                fill=float(c),
                base=0,
                pattern=[[-1, P]],
                channel_multiplier=1,
            )
            sids[c] = t

        X1 = big_p.tile([P, nblk * in_dim], F32, tag="X1")
        X2 = big_p.tile([P, nblk * in_dim], F32, tag="X2")
        OUT = big_p.tile([P, nblk * out_dim], F32, tag="OUT")
        X1r = X1[:].rearrange("p (blk f) -> p blk f", blk=nblk)
        X2r = X2[:].rearrange("p (blk f) -> p blk f", blk=nblk)
        OUTr = OUT[:].rearrange("p (blk f) -> p blk f", blk=nblk)

        # stream inputs per (pass, column-group); products of a group can
        # start as soon as both its X1/X2 chunks land
        for ps in range(n_passes):
            rows = slice(ps * 2 * P, (ps + 1) * 2 * P)
            for g in range(n_groups):
                cols = slice(g * gcols, min((g + 1) * gcols, in_dim))
                nc.sync.dma_start(
                    out=X1r[:, 2 * ps : 2 * ps + 2, cols],
                    in_=x1d[rows, cols].rearrange("(blk p) f -> p blk f", p=P),
                )
                nc.sync.dma_start(
                    out=X2r[:, 2 * ps : 2 * ps + 2, cols],
                    in_=x2d[rows, cols].rearrange("(blk p) f -> p blk f", p=P),
                )

        # engine load-balance state (ns of busy time assigned so far)
        eng_busy = {"dve": 0.0, "pool": float(len(cvals)) * _POOL_SETUP_NS}

        for ps in range(n_passes):
            # pair products, split between DVE and Pool
            prods = {}
            for s1, s2 in pair_order:
                pr = prod_p.tile([P, 2 * P], F32, tag="prod")
                if eng_busy["dve"] + _DVE_TT_NS <= eng_busy["pool"] + _POOL_TT_NS:
                    eng, cost = nc.vector, _DVE_TT_NS
                    eng_busy["dve"] += cost
                else:
                    eng, cost = nc.gpsimd, _POOL_TT_NS
                    eng_busy["pool"] += cost
                eng.tensor_tensor(
                    out=pr[:].rearrange("p (b f) -> p b f", b=2),
                    in0=X1r[:, 2 * ps : 2 * ps + 2, s1 * P : (s1 + 1) * P],
                    in1=X2r[:, 2 * ps : 2 * ps + 2, s2 * P : (s2 + 1) * P],
                    op=mybir.AluOpType.mult,
                )
                prods[(s1, s2)] = pr

            # per-slot contiguous accumulation groups into PSUM banks
            banks = {}
            slots_done = [0] * ((n_so + 1) // 2)
            for so in slot_order:
                k, so_l = divmod(so, 2)
                if k not in banks:
                    banks[k] = psum_p.tile([P, 512], F32, tag="bank")
                tl = slot_terms[so]
                for i, (p, c) in enumerate(tl):
                    nc.tensor.matmul(
                        out=banks[k][:, so_l * 256 : so_l * 256 + 256],
                        lhsT=sids[c][:].bitcast(F32R),
                        rhs=prods[p][:].bitcast(F32R),
                        start=(i == 0),
                        stop=(i == len(tl) - 1),
                    )
                slots_done[k] += 1
                n_in_bank = 2 if 2 * k + 1 < n_so else 1
                if slots_done[k] == n_in_bank:
                    # evacuate bank k: PSUM [p, (so_l, blk, ch)] -> OUT slots
                    nc.scalar.copy(
                        out=OUTr[
                            :, 2 * ps : 2 * ps + 2, 2 * k * P : (2 * k + n_in_bank) * P
                        ].rearrange("p b (s f) -> p s b f", s=n_in_bank),
                        in_=banks[k][:, : n_in_bank * 256].rearrange(
                            "p (s b f) -> p s b f", s=n_in_bank, b=2
                        ),
                    )
                    # store this bank's output columns for both row-blocks
                    nc.sync.dma_start(
                        out=outd[
                            ps * 2 * P : (ps + 1) * 2 * P,
                            2 * k * P : (2 * k + n_in_bank) * P,
                        ].rearrange("(blk p) f -> p blk f", p=P),
                        in_=OUTr[
                            :, 2 * ps : 2 * ps + 2, 2 * k * P : (2 * k + n_in_bank) * P
                        ],
                    )

    nc.finalize()  # run the bacc pass pipeline (wait splitting, regalloc, ...)
    return nc


def kernel(x1, x2, cg_tilde, repids_in1, repids_in2, repids_out, out_dim):
    x1 = np.ascontiguousarray(np.asarray(x1, dtype=np.float32))
    x2 = np.ascontiguousarray(np.asarray(x2, dtype=np.float32))
    cg = np.asarray(cg_tilde, dtype=np.float32)
    r1 = np.asarray(repids_in1).astype(np.int64)
    r2 = np.asarray(repids_in2).astype(np.int64)
    ro = np.asarray(repids_out).astype(np.int64)
    out_dim = int(np.asarray(out_dim))

    B, in_dim = x1.shape
    terms = None
    if (
        B % (N_CORES * 2 * P) == 0
        and in_dim % P == 0
        and out_dim % P == 0
        and x2.shape == x1.shape
    ):
        terms = _detect_plan(r1, r2, ro, cg, in_dim, out_dim)
    if terms is None:
        return _numpy_fallback(x1, x2, cg, r1, r2, ro, out_dim)

    b_shard = B // N_CORES
    key = (B, in_dim, out_dim, np.asarray(terms, dtype=np.float64).tobytes())
    nc = _CACHE.get(key)
    if nc is None:
        nc = _build_program(terms, b_shard, in_dim, out_dim)
        _CACHE[key] = nc

    in_maps = [
        {
            "x1": x1[i * b_shard : (i + 1) * b_shard],
            "x2": x2[i * b_shard : (i + 1) * b_shard],
        }
        for i in range(N_CORES)
    ]
    res = run_bass_kernel_spmd(nc, in_maps, core_ids=list(range(N_CORES)))
    return np.concatenate([res.results[i]["out"] for i in range(N_CORES)], axis=0)


# revision 6
# speedup vs baseline: 3.2797x; 3.2797x over previous
"""CG coupler (segment_reduce) Trainium2 kernel.

out[b, ro[t]] += x1[b, r1[t]] * x2[b, r2[t]] * cg[t]   for t in range(T)

The CG index tables produced by the coupler have a rigid structure: T splits
into runs of exactly 128 consecutive indices (the channel dimension) that are
128-aligned in all three tensors, with a constant coefficient per run.  Each
run is therefore one dense slot-level FMA:

    out[:, so*128:(so+1)*128] += c * x1[:, s1*128:...] * x2[:, s2*128:...]

We detect that structure from the runtime index arrays on the host and bake it
into the Bass program.  Per core (batch is data-parallel across 8 cores):

  - inputs stream in per (pass, column-group) so products can start early
  - the distinct (s1,s2) slot products are computed in fp32, split between
    the DVE and Pool engines by a running load-balance
  - per-term scaled-identity matmuls accumulate into PSUM; operands are
    bitcast to float32r, which the PE runs at 1 cycle/row for moving size
    >= 256 (plain fp32 runs at 4 cycles/row)
  - matmuls for one output slot are issued contiguously (start on first,
    stop on last), so no PSUM-zeroing matmuls are needed
  - the Act engine evacuates each PSUM bank to SBUF; the bank's columns are
    then DMA'd straight to DRAM
"""

import sys

for _p in ("/opt/trn_rl_repo",):
    if _p not in sys.path:
        sys.path.insert(0, _p)

from contextlib import ExitStack

import numpy as np

import concourse.bass as bass
import concourse.mybir as mybir
import concourse.tile as tile
from concourse import bacc
from concourse.bass_utils import run_bass_kernel_spmd

N_CORES = 8
P = 128
F32 = mybir.dt.float32
F32R = mybir.dt.float32r

_CACHE: dict = {}


def _detect_plan(r1, r2, ro, cg, in_dim, out_dim):
    """Return list of (s1, s2, so, c) slot terms, or None if the index tables
    don't have the aligned 128-run structure."""
    T = len(cg)
    if T % P != 0 or len(r1) != T or len(r2) != T or len(ro) != T:
        return None
    d1 = np.diff(r1)
    d2 = np.diff(r2)
    do = np.diff(ro)
    brk = np.where(~((d1 == 1) & (d2 == 1) & (do == 1)))[0] + 1
    starts = np.concatenate([[0], brk])
    ends = np.concatenate([brk, [T]])
    if not np.all(ends - starts == P):
        return None
    a0, b0, o0 = r1[starts], r2[starts], ro[starts]
    if (a0 % P).any() or (b0 % P).any() or (o0 % P).any():
        return None
    if a0.max() + P > in_dim or b0.max() + P > in_dim or o0.max() + P > out_dim:
        return None
    cg2 = np.asarray(cg).reshape(-1, P)
    if not np.all(cg2 == cg2[:, :1]):
        return None
    return list(
        zip(
            (a0 // P).tolist(),
            (b0 // P).tolist(),
            (o0 // P).tolist(),
            cg2[:, 0].astype(np.float64).tolist(),
        )
    )


def _numpy_fallback(x1, x2, cg, r1, r2, ro, out_dim):
    out = np.zeros((x1.shape[0], out_dim), dtype=x1.dtype)
    prod = x1[:, r1] * x2[:, r2] * cg[None, :].astype(x1.dtype)
    np.add.at(out, (slice(None), ro), prod)
    return out


# cost-model engine-busy estimates (ns) for one [128, 256] tensor_tensor
_DVE_TT_NS = 327.0
_POOL_TT_NS = 508.0
_POOL_SETUP_NS = 290.0  # memset + affine_select per scaled identity

SLOTS_PER_GROUP = 4  # column-group granularity for input DMA (512 cols)


def _build_program(terms, b_shard, in_dim, out_dim):
    """Build the per-core Bass program. Every core runs the same program on
    its own batch shard (data-parallel, no collectives)."""
    nblk = b_shard // P
    assert nblk % 2 == 0
    n_passes = nblk // 2
    n_so = out_dim // P
    n_s_in = in_dim // P
    n_groups = (n_s_in + SLOTS_PER_GROUP - 1) // SLOTS_PER_GROUP
    gcols = SLOTS_PER_GROUP * P

    def grp(s):
        return s // SLOTS_PER_GROUP

    # distinct (s1, s2) pairs in data-readiness order
    pairs: dict = {}
    for s1, s2, so, c in terms:
        pairs.setdefault((s1, s2), []).append((so, c))
    pair_order = sorted(pairs, key=lambda p: (max(grp(p[0]), grp(p[1])), p))
    pair_ready = {p: i for i, p in enumerate(pair_order)}

    # output slots ordered by when their last pair product becomes available
    slot_terms: dict = {}
    for s1, s2, so, c in terms:
        slot_terms.setdefault(so, []).append(((s1, s2), c))
    slot_order = sorted(
        slot_terms, key=lambda so: (max(pair_ready[p] for p, _ in slot_terms[so]), so)
    )

    cvals = sorted({c for _, _, _, c in terms})

    nc = bacc.Bacc("TRN2", target_bir_lowering=False, debug=False)
    x1d = nc.dram_tensor("x1", [b_shard, in_dim], F32, kind="ExternalInput").ap()
    x2d = nc.dram_tensor("x2", [b_shard, in_dim], F32, kind="ExternalInput").ap()
    outd = nc.dram_tensor("out", [b_shard, out_dim], F32, kind="ExternalOutput").ap()

    with tile.TileContext(nc) as tc, ExitStack() as ctx:
        const_p = ctx.enter_context(tc.tile_pool(name="const", bufs=1))
        big_p = ctx.enter_context(tc.tile_pool(name="big", bufs=1))
        prod_p = ctx.enter_context(tc.tile_pool(name="prod", bufs=80))
        psum_p = ctx.enter_context(tc.tile_pool(name="psum", bufs=8, space="PSUM"))

        # one fp32 unit identity (Pool), then per-coefficient scaled f32r
        # copies via Act (gpsimd can't legally write f32r; Act rounds)
        ident = const_p.tile([P, P], F32, tag="ident")
        nc.gpsimd.memset(ident[:], 0.0)
        nc.gpsimd.affine_select(
            out=ident[:],
            in_=ident[:],
            compare_op=mybir.AluOpType.not_equal,
            fill=1.0,
            base=0,
            pattern=[[-1, P]],
            channel_multiplier=1,
        )
        sids = {}
        for i, c in enumerate(cvals):
            t = const_p.tile([P, P], F32R, tag=f"sid{i}")
            nc.scalar.activation(
                out=t[:],
                in_=ident[:],
                func=mybir.ActivationFunctionType.Copy,
                scale=float(c),
            )
            sids[c] = t

        X1 = big_p.tile([P, nblk * in_dim], F32, tag="X1")
        X2 = big_p.tile([P, nblk * in_dim], F32, tag="X2")
        OUT = big_p.tile([P, nblk * out_dim], F32, tag="OUT")
        X1r = X1[:].rearrange("p (blk f) -> p blk f", blk=nblk)
        X2r = X2[:].rearrange("p (blk f) -> p blk f", blk=nblk)
        OUTr = OUT[:].rearrange("p (blk f) -> p blk f", blk=nblk)

        # stream inputs per (pass, column-group); products of a group can
        # start as soon as both its X1/X2 chunks land
        for ps in range(n_passes):
            rows = slice(ps * 2 * P, (ps + 1) * 2 * P)
            for g in range(n_groups):
                cols = slice(g * gcols, min((g + 1) * gcols, in_dim))
                nc.sync.dma_start(
                    out=X1r[:, 2 * ps : 2 * ps + 2, cols],
                    in_=x1d[rows, cols].rearrange("(blk p) f -> p blk f", p=P),
                )
                nc.sync.dma_start(
                    out=X2r[:, 2 * ps : 2 * ps + 2, cols],
                    in_=x2d[rows, cols].rearrange("(blk p) f -> p blk f", p=P),
                )

        # engine load-balance state (ns of busy time assigned so far)
        eng_busy = {"dve": 0.0, "pool": _POOL_SETUP_NS}

        for ps in range(n_passes):
            # pair products, split between DVE and Pool
            prods = {}
            for s1, s2 in pair_order:
                pr = prod_p.tile([P, 2 * P], F32R, tag="prod")
                if eng_busy["dve"] + _DVE_TT_NS <= eng_busy["pool"] + _POOL_TT_NS:
                    eng, cost = nc.vector, _DVE_TT_NS
                    eng_busy["dve"] += cost
                else:
                    eng, cost = nc.gpsimd, _POOL_TT_NS
                    eng_busy["pool"] += cost
                eng.tensor_tensor(
                    out=pr[:].rearrange("p (b f) -> p b f", b=2),
                    in0=X1r[:, 2 * ps : 2 * ps + 2, s1 * P : (s1 + 1) * P],
                    in1=X2r[:, 2 * ps : 2 * ps + 2, s2 * P : (s2 + 1) * P],
                    op=mybir.AluOpType.mult,
                )
                prods[(s1, s2)] = pr

            # per-slot contiguous accumulation groups into PSUM banks
            banks = {}
            slots_done = [0] * ((n_so + 1) // 2)
            for so in slot_order:
                k, so_l = divmod(so, 2)
                if k not in banks:
                    bk = psum_p.tile([P, 512], F32, tag="bank")
                    banks[k] = bk
                tl = slot_terms[so]
                for i, (p, c) in enumerate(tl):
                    nc.tensor.matmul(
                        out=banks[k][:, so_l * 256 : so_l * 256 + 256],
                        lhsT=sids[c][:],
                        rhs=prods[p][:],
                        start=(i == 0),
                        stop=(i == len(tl) - 1),
                    )
                slots_done[k] += 1
                n_in_bank = 2 if 2 * k + 1 < n_so else 1
                if slots_done[k] == n_in_bank:
                    # evacuate bank k: PSUM [p, (so_l, blk, ch)] -> OUT slots
                    nc.scalar.copy(
                        out=OUTr[
                            :, 2 * ps : 2 * ps + 2, 2 * k * P : (2 * k + n_in_bank) * P
                        ].rearrange("p b (s f) -> p s b f", s=n_in_bank),
                        in_=banks[k][:, : n_in_bank * 256].rearrange(
                            "p (s b f) -> p s b f", s=n_in_bank, b=2
                        ),
                    )
                    # store this bank's output columns for both row-blocks
                    nc.sync.dma_start(
                        out=outd[
                            ps * 2 * P : (ps + 1) * 2 * P,
                            2 * k * P : (2 * k + n_in_bank) * P,
                        ].rearrange("(blk p) f -> p blk f", p=P),
                        in_=OUTr[
                            :, 2 * ps : 2 * ps + 2, 2 * k * P : (2 * k + n_in_bank) * P
                        ],
                    )

    nc.finalize()  # run the bacc pass pipeline (wait splitting, regalloc, ...)
    return nc


def kernel(x1, x2, cg_tilde, repids_in1, repids_in2, repids_out, out_dim):
    x1 = np.ascontiguousarray(np.asarray(x1, dtype=np.float32))
    x2 = np.ascontiguousarray(np.asarray(x2, dtype=np.float32))
    cg = np.asarray(cg_tilde, dtype=np.float32)
    r1 = np.asarray(repids_in1).astype(np.int64)
    r2 = np.asarray(repids_in2).astype(np.int64)
    ro = np.asarray(repids_out).astype(np.int64)
    out_dim = int(np.asarray(out_dim))

    B, in_dim = x1.shape
    terms = None
    if (
        B % (N_CORES * 2 * P) == 0
        and in_dim % P == 0
        and out_dim % P == 0
        and x2.shape == x1.shape
    ):
        terms = _detect_plan(r1, r2, ro, cg, in_dim, out_dim)
    if terms is None:
        return _numpy_fallback(x1, x2, cg, r1, r2, ro, out_dim)

    b_shard = B // N_CORES
    key = (B, in_dim, out_dim, np.asarray(terms, dtype=np.float64).tobytes())
    nc = _CACHE.get(key)
    if nc is None:
        nc = _build_program(terms, b_shard, in_dim, out_dim)
        _CACHE[key] = nc

    in_maps = [
        {
            "x1": x1[i * b_shard : (i + 1) * b_shard],
            "x2": x2[i * b_shard : (i + 1) * b_shard],
        }
        for i in range(N_CORES)
    ]
    res = run_bass_kernel_spmd(nc, in_maps, core_ids=list(range(N_CORES)))
    return np.concatenate([res.results[i]["out"] for i in range(N_CORES)], axis=0)


# revision 8
# speedup vs baseline: 3.4956x; 1.0658x over previous
"""CG coupler (segment_reduce) Trainium2 kernel.

out[b, ro[t]] += x1[b, r1[t]] * x2[b, r2[t]] * cg[t]   for t in range(T)

The CG index tables produced by the coupler have a rigid structure: T splits
into runs of exactly 128 consecutive indices (the channel dimension) that are
128-aligned in all three tensors, with a constant coefficient per run.  Each
run is therefore one dense slot-level FMA:

    out[:, so*128:(so+1)*128] += c * x1[:, s1*128:...] * x2[:, s2*128:...]

We detect that structure from the runtime index arrays on the host and bake it
into the Bass program.  Per core (batch is data-parallel across 8 cores):

  - inputs stream in per (pass, column-group) so products can start early
  - the distinct (s1,s2) slot products are computed in fp32, split between
    the DVE and Pool engines by a running load-balance
  - per-term scaled-identity matmuls accumulate into PSUM; operands are
    bitcast to float32r, which the PE runs at 1 cycle/row for moving size
    >= 256 (plain fp32 runs at 4 cycles/row)
  - matmuls for one output slot are issued contiguously (start on first,
    stop on last), so no PSUM-zeroing matmuls are needed
  - the Act engine evacuates each PSUM bank to SBUF; the bank's columns are
    then DMA'd straight to DRAM
"""

import sys

for _p in ("/opt/trn_rl_repo",):
    if _p not in sys.path:
        sys.path.insert(0, _p)

from contextlib import ExitStack

import numpy as np

import concourse.bass as bass
import concourse.mybir as mybir
import concourse.tile as tile
from concourse import bacc
from concourse.bass_utils import run_bass_kernel_spmd

N_CORES = 8
P = 128
F32 = mybir.dt.float32
F32R = mybir.dt.float32r

_CACHE: dict = {}


def _detect_plan(r1, r2, ro, cg, in_dim, out_dim):
    """Return list of (s1, s2, so, c) slot terms, or None if the index tables
    don't have the aligned 128-run structure."""
    T = len(cg)
    if T % P != 0 or len(r1) != T or len(r2) != T or len(ro) != T:
        return None
    d1 = np.diff(r1)
    d2 = np.diff(r2)
    do = np.diff(ro)
    brk = np.where(~((d1 == 1) & (d2 == 1) & (do == 1)))[0] + 1
    starts = np.concatenate([[0], brk])
    ends = np.concatenate([brk, [T]])
    if not np.all(ends - starts == P):
        return None
    a0, b0, o0 = r1[starts], r2[starts], ro[starts]
    if (a0 % P).any() or (b0 % P).any() or (o0 % P).any():
        return None
    if a0.max() + P > in_dim or b0.max() + P > in_dim or o0.max() + P > out_dim:
        return None
    cg2 = np.asarray(cg).reshape(-1, P)
    if not np.all(cg2 == cg2[:, :1]):
        return None
    return list(
        zip(
            (a0 // P).tolist(),
            (b0 // P).tolist(),
            (o0 // P).tolist(),
            cg2[:, 0].astype(np.float64).tolist(),
        )
    )


def _numpy_fallback(x1, x2, cg, r1, r2, ro, out_dim):
    out = np.zeros((x1.shape[0], out_dim), dtype=x1.dtype)
    prod = x1[:, r1] * x2[:, r2] * cg[None, :].astype(x1.dtype)
    np.add.at(out, (slice(None), ro), prod)
    return out


# cost-model engine-busy estimates (ns) for one [128, 256] tensor_tensor
_DVE_TT_NS = 327.0
_POOL_TT_NS = 508.0
_POOL_SETUP_NS = 290.0  # memset + affine_select per scaled identity

SLOTS_PER_GROUP = 4  # column-group granularity for input DMA (512 cols)


def _build_program(terms, b_shard, in_dim, out_dim):
    """Build the per-core Bass program. Every core runs the same program on
    its own batch shard (data-parallel, no collectives)."""
    nblk = b_shard // P
    assert nblk % 2 == 0
    n_passes = nblk // 2
    n_so = out_dim // P
    n_s_in = in_dim // P

    # load chunking: fine-grained leading chunks so the first pair products
    # (low slots) can start ~2us in, coarser after
    def pass_chunks(ps):
        if ps == 0:
            singles = min(4, n_s_in)
            chunks = [[s] for s in range(singles)]
            s = singles
        else:
            chunks, s = [], 0
        while s < n_s_in:
            e = min(s + SLOTS_PER_GROUP, n_s_in)
            chunks.append(list(range(s, e)))
            s = e
        return chunks

    # estimated DMA completion time per (pass, slot) assuming serial DMA
    # engines at ~0.36 B/ns starting ~1.4us in (HWDGE pipeline fill)
    load_done = {}
    t = 1400.0
    for ps in range(n_passes):
        for chunk in pass_chunks(ps):
            dur = 2 * P * len(chunk) * P * 4 / 0.36  # both row-blocks, ns
            t += dur  # x1 chunk
            t += dur  # x2 chunk
            for s in chunk:
                load_done[(ps, s)] = t

    # distinct (s1, s2) pairs; per-pass greedy engine assignment by
    # projected completion time
    pairs: dict = {}
    for s1, s2, so, c in terms:
        pairs.setdefault((s1, s2), []).append((so, c))

    slot_of = {}  # (pass, pair) -> terms ordering comes later
    cvals_first_use = {}

    nc = bacc.Bacc("TRN2", target_bir_lowering=False, debug=False)
    x1d = nc.dram_tensor("x1", [b_shard, in_dim], F32, kind="ExternalInput").ap()
    x2d = nc.dram_tensor("x2", [b_shard, in_dim], F32, kind="ExternalInput").ap()
    outd = nc.dram_tensor("out", [b_shard, out_dim], F32, kind="ExternalOutput").ap()

    with tile.TileContext(nc) as tc, ExitStack() as ctx:
        const_p = ctx.enter_context(tc.tile_pool(name="const", bufs=1))
        big_p = ctx.enter_context(tc.tile_pool(name="big", bufs=1))
        prod_p = ctx.enter_context(tc.tile_pool(name="prod", bufs=88))
        psum_p = ctx.enter_context(tc.tile_pool(name="psum", bufs=8, space="PSUM"))

        # fp32 unit identity (Pool). Scaled f32r copies are made on Act below,
        # ordered by first use (gpsimd can't legally write f32r; Act rounds).
        ident = const_p.tile([P, P], F32, tag="ident")
        nc.gpsimd.memset(ident[:], 0.0)
        nc.gpsimd.affine_select(
            out=ident[:],
            in_=ident[:],
            compare_op=mybir.AluOpType.not_equal,
            fill=1.0,
            base=0,
            pattern=[[-1, P]],
            channel_multiplier=1,
        )

        X1 = big_p.tile([P, nblk * in_dim], F32, tag="X1")
        X2 = big_p.tile([P, nblk * in_dim], F32, tag="X2")
        OUT = big_p.tile([P, nblk * out_dim], F32, tag="OUT")
        X1r = X1[:].rearrange("p (blk f) -> p blk f", blk=nblk)
        X2r = X2[:].rearrange("p (blk f) -> p blk f", blk=nblk)
        OUTr = OUT[:].rearrange("p (blk f) -> p blk f", blk=nblk)

        for ps in range(n_passes):
            rows = slice(ps * 2 * P, (ps + 1) * 2 * P)
            for chunk in pass_chunks(ps):
                cols = slice(chunk[0] * P, (chunk[-1] + 1) * P)
                nc.sync.dma_start(
                    out=X1r[:, 2 * ps : 2 * ps + 2, cols],
                    in_=x1d[rows, cols].rearrange("(blk p) f -> p blk f", p=P),
                )
                nc.sync.dma_start(
                    out=X2r[:, 2 * ps : 2 * ps + 2, cols],
                    in_=x2d[rows, cols].rearrange("(blk p) f -> p blk f", p=P),
                )

        # plan products and term order for every pass up front (host-side),
        # so scaled identities can be built in first-use order
        eng_vt = {"dve": 0.0, "pool": _POOL_SETUP_NS}
        plan = []  # per pass: (prod_assign list, term list)
        for ps in range(n_passes):
            ready = {
                p: max(load_done[(ps, p[0])], load_done[(ps, p[1])]) for p in pairs
            }
            order = sorted(pairs, key=lambda p: (ready[p], p))
            assign = []
            t_done = {}
            for p in order:
                fin_d = max(ready[p], eng_vt["dve"]) + _DVE_TT_NS
                fin_p = max(ready[p], eng_vt["pool"]) + _POOL_TT_NS
                if fin_d <= fin_p:
                    eng_vt["dve"] = fin_d
                    assign.append((p, "dve"))
                    t_done[p] = fin_d
                else:
                    eng_vt["pool"] = fin_p
                    assign.append((p, "pool"))
                    t_done[p] = fin_p
            term_list = []  # (t_done, so, pair, c)
            for (s1, s2), tl in pairs.items():
                for so, c in tl:
                    term_list.append((t_done[(s1, s2)], so, (s1, s2), c))
            term_list.sort()
            plan.append((assign, term_list))
            for _, so, p, c in term_list:
                cvals_first_use.setdefault(c, len(cvals_first_use))

        # scaled f32r identities on Act, in first-use order
        sids = {}
        for c, i in sorted(cvals_first_use.items(), key=lambda kv: kv[1]):
            t_ = const_p.tile([P, P], F32R, tag=f"sid{i}")
            nc.scalar.activation(
                out=t_[:],
                in_=ident[:],
                func=mybir.ActivationFunctionType.Copy,
                scale=float(c),
            )
            sids[c] = t_

        n_banks = (n_so + 1) // 2

        for ps in range(n_passes):
            assign, term_list = plan[ps]

            banks = []
            for k in range(n_banks):
                bk = psum_p.tile([P, 512], F32, tag="bank")
                banks.append(bk)

            prods = {}
            for p, eng_name in assign:
                pr = prod_p.tile([P, 2 * P], F32R, tag="prod")
                eng = nc.vector if eng_name == "dve" else nc.gpsimd
                eng.tensor_tensor(
                    out=pr[:].rearrange("p (b f) -> p b f", b=2),
                    in0=X1r[:, 2 * ps : 2 * ps + 2, p[0] * P : (p[0] + 1) * P],
                    in1=X2r[:, 2 * ps : 2 * ps + 2, p[1] * P : (p[1] + 1) * P],
                    op=mybir.AluOpType.mult,
                )
                prods[p] = pr

            # contiguous per-slot accumulation groups, slots ordered by the
            # estimated completion time of their last product
            slot_key = {}
            for td, so, p, c in term_list:
                slot_key[so] = max(slot_key.get(so, 0.0), td)
            slot_terms = {}
            for td, so, p, c in term_list:
                slot_terms.setdefault(so, []).append((td, p, c))
            n_in_bank_done = [0] * n_banks
            for so in sorted(slot_terms, key=lambda s: (slot_key[s], s)):
                k, so_l = divmod(so, 2)
                tl = sorted(slot_terms[so])
                for i, (_, p, c) in enumerate(tl):
                    nc.tensor.matmul(
                        out=banks[k][:, so_l * 256 : so_l * 256 + 256],
                        lhsT=sids[c][:],
                        rhs=prods[p][:],
                        start=(i == 0),
                        stop=(i == len(tl) - 1),
                    )
                n_in_bank_done[k] += 1
                if n_in_bank_done[k] == (2 if 2 * k + 1 < n_so else 1):
                    n_in_bank = 2 if 2 * k + 1 < n_so else 1
                    nc.scalar.copy(
                        out=OUTr[
                            :, 2 * ps : 2 * ps + 2, 2 * k * P : (2 * k + n_in_bank) * P
                        ].rearrange("p b (s f) -> p s b f", s=n_in_bank),
                        in_=banks[k][:, : n_in_bank * 256].rearrange(
                            "p (s b f) -> p s b f", s=n_in_bank, b=2
                        ),
                    )
                    nc.sync.dma_start(
                        out=outd[
                            ps * 2 * P : (ps + 1) * 2 * P,
                            2 * k * P : (2 * k + n_in_bank) * P,
                        ].rearrange("(blk p) f -> p blk f", p=P),
                        in_=OUTr[
                            :, 2 * ps : 2 * ps + 2, 2 * k * P : (2 * k + n_in_bank) * P
                        ],
                    )

    nc.finalize()  # run the bacc pass pipeline (wait splitting, regalloc, ...)
    return nc


def kernel(x1, x2, cg_tilde, repids_in1, repids_in2, repids_out, out_dim):
    x1 = np.ascontiguousarray(np.asarray(x1, dtype=np.float32))
    x2 = np.ascontiguousarray(np.asarray(x2, dtype=np.float32))
    cg = np.asarray(cg_tilde, dtype=np.float32)
    r1 = np.asarray(repids_in1).astype(np.int64)
    r2 = np.asarray(repids_in2).astype(np.int64)
    ro = np.asarray(repids_out).astype(np.int64)
    out_dim = int(np.asarray(out_dim))

    B, in_dim = x1.shape
    terms = None
    if (
        B % (N_CORES * 2 * P) == 0
        and in_dim % P == 0
        and out_dim % P == 0
        and x2.shape == x1.shape
    ):
        terms = _detect_plan(r1, r2, ro, cg, in_dim, out_dim)
    if terms is None:
        return _numpy_fallback(x1, x2, cg, r1, r2, ro, out_dim)

    b_shard = B // N_CORES
    key = (B, in_dim, out_dim, np.asarray(terms, dtype=np.float64).tobytes())
    nc = _CACHE.get(key)
    if nc is None:
        nc = _build_program(terms, b_shard, in_dim, out_dim)
        _CACHE[key] = nc

    in_maps = [
        {
            "x1": x1[i * b_shard : (i + 1) * b_shard],
            "x2": x2[i * b_shard : (i + 1) * b_shard],
        }
        for i in range(N_CORES)
    ]
    res = run_bass_kernel_spmd(nc, in_maps, core_ids=list(range(N_CORES)))
    return np.concatenate([res.results[i]["out"] for i in range(N_CORES)], axis=0)
